# revision 1
# baseline (speedup 1.0000x reference)
"""Trainium2 Bass kernel for nn_Attention_82660940579436.

Computation (see reference):
    q     = mean_s(hidden @ Wq.T + bq)            [B, H]
    key   = tanh(hidden @ Wk.T + bk)              [S, B, H]
    score = einsum('bsh,bh->bs', key, q) + mask   [B, S]
    out   = softmax(score) @ key                  [B, H]

Sharding: data-parallel over batch. B=32 over 8 cores -> 4 batches/core.
Each core streams its 32 MiB hidden slice once, keeps key resident in SBUF
as bf16, then does a second SBUF-only pass for the softmax-weighted sum.

Device algorithm per core (4 local batches, tokens = (s, b) pairs):
  Phase A (per 128-token tile: 32 s-positions x 4 batches):
    - DMA hidden tile [128 tok, 512] fp32
    - PE transpose -> hT [512 j, 128 tok] (4x 128x128 via identity)
    - DVE copy hT PSUM->SBUF; DVE strided reduce accumulates sum_s(h) per (j,b)
    - PE: z = bk (rank-1 matmul) + hT.T @ WkT (4 matmuls, fp32) -> PSUM
    - ACT: key = tanh(z) -> resident SBUF bf16
  q = (sum_s h / S) @ WqT + bq  (tiny matmuls); qrep[p,:] = q[p%4,:] via PE
  Phase B (per tile):
    - DVE mul + reduce: score[p] = sum_i key[p,i]*qrep[p,i]
      (tensor_tensor_reduce would fuse this, but custom DVE ops fault at
      runtime under the axon compile path -- ucode tables are not shipped)
    - ACT: e = exp(score + mask_bias)  (mask as per-partition bias)
    - DVE: e_ind[p,g] = e[p] * (p%4==g)
    - PE: numer[4,512] += e_ind.T @ key ; den[4,1] += e_ind.T @ ones
  out = numer / den  -> DMA out [4, 512]

All big-matmul operands are bf16: TRN2's PE runs fp32 matmuls at 1/4 rate
(two half-speed passes), bf16 at 1 column/cycle. The hidden input is cast
fp32->bf16 during the SWDGE DMA load (free). The q path stays fp32.
Cost-model timeline (concourse InstructionCostModel): ~330 us/core.

exp() needs no max-subtraction: scores are O(1) by construction, masked
positions get -60 bias -> exp underflows to ~1e-27 (reference's -10000
mask likewise produces exact zeros after its own softmax).

All constants ship in two packed tensors (one fp32, one bf16) loaded by a
single DMA each, and two dummy PE ops observe those DMA lanes up front:
walrus only allows ONE sync-wait on a Matmult (S3_LW), so every real
matmul must have at most one not-yet-observed dependency.
"""

import sys
from contextlib import ExitStack

import numpy as np

if "/opt/trn_rl_repo" not in sys.path:
    sys.path.insert(0, "/opt/trn_rl_repo")

import ml_dtypes  # noqa: E402

import concourse.bacc as bacc  # noqa: E402
import concourse.bass as bass  # noqa: E402
import concourse.mybir as mybir  # noqa: E402
import concourse.tile as tile  # noqa: E402
from concourse.bass_utils import run_bass_kernel_spmd  # noqa: E402

S, B, H = 4096, 32, 512
NCORES = 8
BPC = B // NCORES  # 4 batches per core
NT = 128  # tiles per core
SS = S // NT  # 32 s-positions per tile
TOK = SS * BPC  # 128 tokens per tile
HC = H // 128  # 4 chunks of the H (j / i) dims
MASK_NEG = -60.0
F32 = mybir.dt.float32
BF16 = mybir.dt.bfloat16
AF = mybir.ActivationFunctionType
ALU = mybir.AluOpType
BF16NP = ml_dtypes.bfloat16

# fp32 const pack layout (offsets in fp32 elements, [128, PACKF] tensor)
OFF_WQ = 0  # [128, 2048] WqT chunks
OFF_MASK = 2048  # [128, 128] mask bias (0 / MASK_NEG), col=tile
OFF_BQ = 2176  # [4, 512] bq rows
OFF_IND4T = 2688  # [4, 128] indicator transposed
OFF_ZERO = 2816  # [128, 1] zeros (tanh bias)
PACKF = 2824
# bf16 const pack layout ([128, PACKB]) — matmul operands live here:
# fp32 matmuls run at 1/4 rate on TRN2, bf16 at full rate.
OFFB_WK = 0  # [128, 2048] WkT chunks
OFFB_ID = 2048  # [128, 128] identity
OFFB_BK = 2176  # [1, 512] bk on partition 0
OFFB_ONESROW = 2688  # [1, 128] ones on partition 0
OFFB_IND4 = 2816  # [128, 4] indicator
OFFB_ONES = 2820  # [128, 1] ones
PACKB = 2824

# tuning knobs (read at build time)
KNOBS = {
    "h_bufs": 8,
    "hT_bufs": 3,
    "hTps_bufs": 2,
    "keyps_bufs": 2,
    "small_bufs": 3,
    "phase_a_only": False,
    "no_transpose": False,  # debug: skip transposes (wrong results)
    "no_phase_b_mm": False,
}


def _build_kernel_body(tc, aps):
    nc = tc.nc
    x, packf, packb, y = aps["x"], aps["packf"], aps["packb"], aps["y"]

    with ExitStack() as ctx:
        consts = ctx.enter_context(tc.tile_pool(name="consts", bufs=1))
        ph = ctx.enter_context(tc.tile_pool(name="h", bufs=KNOBS["h_bufs"]))
        phT = ctx.enter_context(tc.tile_pool(name="hT", bufs=KNOBS["hT_bufs"]))
        pkeys = ctx.enter_context(tc.tile_pool(name="keys", bufs=NT))
        psmall = ctx.enter_context(tc.tile_pool(name="small", bufs=KNOBS["small_bufs"]))
        pacc = ctx.enter_context(tc.tile_pool(name="acc", bufs=1))
        pps_hT = ctx.enter_context(tc.tile_pool(name="ps_hT", bufs=KNOBS["hTps_bufs"], space="PSUM"))
        pps_key = ctx.enter_context(tc.tile_pool(name="ps_key", bufs=KNOBS["keyps_bufs"], space="PSUM"))
        pps_acc = ctx.enter_context(tc.tile_pool(name="ps_acc", bufs=1, space="PSUM"))
        pps_sm = ctx.enter_context(tc.tile_pool(name="ps_sm", bufs=1, space="PSUM"))

        # ---- constants: one DMA per pack ----
        cf = consts.tile([128, PACKF], F32)
        nc.sync.dma_start(cf, packf)
        cb = consts.tile([128, PACKB], BF16)
        nc.sync.dma_start(cb, packb)

        def wk_sb(c):
            return cb[:, OFFB_WK + c * 512 : OFFB_WK + (c + 1) * 512]

        def wq_sb(c):
            return cf[:, OFF_WQ + c * 512 : OFF_WQ + (c + 1) * 512]

        id_sb = cb[:, OFFB_ID : OFFB_ID + 128]
        maskb_sb = cf[:, OFF_MASK : OFF_MASK + NT]
        bk_sb = cb[0:1, OFFB_BK : OFFB_BK + H]
        bq_sb = cf[0:BPC, OFF_BQ : OFF_BQ + H]
        ones_row_sb = cb[0:1, OFFB_ONESROW : OFFB_ONESROW + 128]
        ind4T_sb = cf[0:BPC, OFF_IND4T : OFF_IND4T + 128]
        zero_sb = cf[:, OFF_ZERO : OFF_ZERO + 1]
        ind4_sb = cb[:, OFFB_IND4 : OFFB_IND4 + BPC]
        ones1_sb = cb[:, OFFB_ONES : OFFB_ONES + 1]

        # Dummy PE ops: observe each const-pack DMA lane once, so no real
        # matmul ever needs two sync-waits (walrus S3_LW limit is one).
        scr = pps_sm.tile([128, H], BF16, tag="smb")
        nc.tensor.transpose(scr[:, :128], id_sb, id_sb)
        scr2 = pps_sm.tile([128, H], F32, tag="sm")
        nc.tensor.matmul(scr2[:128, :128], ind4T_sb, wq_sb(0)[0:BPC, 0:128],
                         start=True, stop=True)

        macc = pacc.tile([128, HC * BPC], F32)  # sum_s h, laid out (j_local, (c, g))
        nc.vector.memset(macc, 0.0)

        # ---- Phase A ----
        keys = []
        for t in range(NT):
            h_t = ph.tile([TOK, H], BF16, tag="h")
            # SWDGE (gpsimd): casts fp32->bf16 during the DMA (free), and its
            # ucode path tolerates the multi-sync-waits this load needs.
            nc.gpsimd.dma_start(h_t, x[t])

            hT_ps = pps_hT.tile([128, H], BF16, tag="hT")
            for c in range(HC):
                nc.tensor.transpose(
                    hT_ps[:, c * 128 : (c + 1) * 128],
                    h_t[:, c * 128 : (c + 1) * 128],
                    id_sb,
                )
            hT_sb = phT.tile([128, H], BF16, tag="hT_sb")
            nc.vector.tensor_copy(hT_sb, hT_ps)

            red = psmall.tile([128, HC * BPC], F32, tag="red")
            nc.vector.tensor_reduce(
                red,
                hT_sb.rearrange("p (c s g) -> p c g s", c=HC, s=SS, g=BPC),
                axis=mybir.AxisListType.X,
                op=ALU.add,
            )
            nc.vector.tensor_add(macc, macc, red)

            key_ps = pps_key.tile([TOK, H], F32, tag="key")
            nc.tensor.matmul(key_ps, ones_row_sb, bk_sb, start=True, stop=False)
            for c in range(HC):
                nc.tensor.matmul(
                    key_ps,
                    hT_sb[:, c * 128 : (c + 1) * 128],
                    wk_sb(c),
                    start=False,
                    stop=(c == HC - 1),
                )
            key_t = pkeys.tile([TOK, H], BF16, tag="key")
            nc.scalar.activation(key_t, key_ps, AF.Tanh, bias=zero_sb)
            keys.append(key_t)

        # ---- q = (sum_s h / S) @ WqT + bq ; qrep[p] = q[p%4] ----
        q_ps = pps_sm.tile([BPC, H], F32, tag="sm")
        for c in range(HC):
            nc.tensor.matmul(
                q_ps,
                macc[:, c * BPC : (c + 1) * BPC],
                wq_sb(c),
                start=(c == 0),
                stop=(c == HC - 1),
            )
        q_sb = pacc.tile([BPC, H], F32)
        nc.scalar.mul(q_sb, q_ps, 1.0 / S)
        nc.vector.tensor_add(q_sb, q_sb, bq_sb)
        qrep_ps = pps_sm.tile([128, H], F32, tag="sm")
        nc.tensor.matmul(qrep_ps, ind4T_sb, q_sb, start=True, stop=True)
        qrep_sb = pacc.tile([128, H], BF16)
        nc.scalar.copy(qrep_sb, qrep_ps)

        # ---- Phase B ----
        numer_ps = pps_acc.tile([BPC, H], F32, tag="numer")
        den_ps = pps_acc.tile([BPC, 1], F32, tag="den")
        for t in range(NT):
            # Score products split 40/60 between DVE and the otherwise-idle
            # GPSIMD engine; the X-axis reduce is DVE-only. (TimelineSim:
            # 330 -> 306 us; all-DVE and all-GPSIMD are both worse.)
            if t % 5 < 2:
                prod = psmall.tile([TOK, H], BF16, tag="prod")
                nc.vector.tensor_mul(prod, keys[t], qrep_sb)
            else:
                prod = psmall.tile([TOK, H], BF16, tag="prodg")
                nc.gpsimd.tensor_mul(prod, keys[t], qrep_sb)
            sc_t = psmall.tile([TOK, 1], F32, tag="sc")
            # The row-sum reduce also splits across engines: tensor_reduce on
            # DVE for half the tiles, ACT's activation(Copy, accum_out=) for
            # the other half (ACT is mostly idle in phase B). 306 -> 290 us.
            if t % 2 == 0:
                nc.vector.tensor_reduce(
                    sc_t, prod, axis=mybir.AxisListType.X, op=ALU.add
                )
            else:
                pc = psmall.tile([TOK, H], BF16, tag="pc")
                nc.scalar.activation(pc, prod, AF.Copy, accum_out=sc_t)
            e_t = psmall.tile([TOK, 1], F32, tag="e")
            nc.scalar.activation(e_t, sc_t, AF.Exp, bias=maskb_sb[:, t : t + 1])
            ei_t = psmall.tile([TOK, BPC], BF16, tag="ei")
            nc.vector.tensor_scalar_mul(ei_t, ind4_sb, e_t)
            nc.tensor.matmul(
                numer_ps, ei_t, keys[t], start=(t == 0), stop=(t == NT - 1)
            )
            nc.tensor.matmul(
                den_ps, ei_t, ones1_sb, start=(t == 0), stop=(t == NT - 1)
            )

        # ---- out = numer / den ----
        rcp = pacc.tile([BPC, 1], F32)
        nc.vector.reciprocal(rcp, den_ps)
        out_sb = pacc.tile([BPC, H], F32)
        nc.vector.tensor_scalar_mul(out_sb, numer_ps, rcp)
        nc.sync.dma_start(y, out_sb)


_CACHE = {}


def _fix_dma_waits(nc):
    """walrus's DMA_DIRECT2D lowering only has ONE sync-wait slot, but Tile
    gives each hidden-tile load two waits: (a) WAR, engine sem, readers of the
    recycled buffer; (b) WAW, DMA-lane sem, the load that wrote this buffer 8
    tiles ago.  All these loads sit on the single SWDGE queue (qPoolDynamic):
    descriptor generation is program-ordered and each SDMA engine drains its
    ring FIFO, and a given SBUF byte always belongs to the same engine, so
    same-buffer writes from this queue cannot reorder -- the WAW wait is
    hardware-redundant.  Drop it; keep the WAR wait.

    Also sanity-check the remaining wait counts against walrus's empirical
    limits (DMACopy: 1, everything else: 2, Drain exempt)."""
    for b in nc.m.functions[0].blocks:
        for i in b.instructions:
            si = i.sync_info
            if si is None:
                continue
            waits = list(si.on_wait)
            if (
                type(i).__name__ == "InstDMACopy"
                and getattr(i, "queue", "") == "qPoolDynamic"
                and len(waits) == 2
            ):
                lane = [w for w in waits if w.ant_name.startswith("DMASW")]
                eng = [w for w in waits if not w.ant_name.startswith("DMA")]
                if len(lane) == 1 and len(eng) == 1:
                    out0 = i.outs[0]
                    name = getattr(getattr(out0, "bass_ap", None), "tensor", None)
                    name = getattr(name, "name", "")
                    if name.startswith("h_t"):
                        si.on_wait = eng
                        continue
            if type(i).__name__ in ("InstDrain", "InstEventSemaphore"):
                continue
            limit = 1 if type(i).__name__ == "InstDMACopy" else 2
            if len(waits) > limit:
                raise RuntimeError(
                    f"{i.name} {type(i).__name__} has {len(waits)} waits "
                    f"(> {limit}): {[(w.ant_name, w.wait_value) for w in waits]}"
                )


def _get_program():
    if "nc" in _CACHE:
        return _CACHE["nc"], _CACHE["aps"]
    nc = bacc.Bacc(None, target_bir_lowering=False, debug=False)
    aps = {
        "x": nc.dram_tensor("x", [NT, TOK, H], F32, kind="ExternalInput").ap(),
        "packf": nc.dram_tensor("packf", [128, PACKF], F32, kind="ExternalInput").ap(),
        "packb": nc.dram_tensor("packb", [128, PACKB], BF16, kind="ExternalInput").ap(),
        "y": nc.dram_tensor("y", [BPC, H], F32, kind="ExternalOutput").ap(),
    }
    with tile.TileContext(nc) as tc:
        _build_kernel_body(tc, aps)
    nc.finalize()  # Bacc.compile: wait legalization (EVSEM splits), LDW moves
    _CACHE["nc"] = nc
    _CACHE["aps"] = aps
    return nc, aps


def _make_in_maps(hidden_states, Wq, bq, Wk, bk, lengths):
    hidden = np.asarray(hidden_states, dtype=np.float32)
    Wq = np.asarray(Wq, dtype=np.float32)
    Wk = np.asarray(Wk, dtype=np.float32)
    bqv = np.asarray(bq, dtype=np.float32)
    bkv = np.asarray(bk, dtype=np.float32)
    lens = np.asarray(lengths).astype(np.int64)

    p = np.arange(128)
    packb = np.zeros((128, PACKB), dtype=BF16NP)
    packb[:, OFFB_WK : OFFB_WK + 2048] = (
        np.ascontiguousarray(Wk.T)
        .reshape(HC, 128, H)
        .transpose(1, 0, 2)
        .reshape(128, 2048)
        .astype(BF16NP)
    )
    packb[:, OFFB_ID : OFFB_ID + 128] = np.eye(128, dtype=BF16NP)
    packb[0, OFFB_BK : OFFB_BK + H] = bkv.astype(BF16NP)
    packb[0, OFFB_ONESROW : OFFB_ONESROW + 128] = BF16NP(1.0)
    packb[:, OFFB_IND4 : OFFB_IND4 + BPC] = (
        p[:, None] % BPC == np.arange(BPC)[None, :]
    ).astype(BF16NP)
    packb[:, OFFB_ONES] = BF16NP(1.0)

    base_packf = np.zeros((128, PACKF), dtype=np.float32)
    base_packf[:, OFF_WQ : OFF_WQ + 2048] = (
        np.ascontiguousarray(Wq.T).reshape(HC, 128, H).transpose(1, 0, 2).reshape(128, 2048)
    )
    base_packf[0:BPC, OFF_BQ : OFF_BQ + H] = bqv[None, :]
    base_packf[0:BPC, OFF_IND4T : OFF_IND4T + 128] = (
        p[None, :] % BPC == np.arange(BPC)[:, None]
    ).astype(np.float32)

    in_maps = []
    s_of_p = p // BPC
    t_idx = np.arange(NT)
    for c in range(NCORES):
        xc = np.ascontiguousarray(hidden[:, c * BPC : (c + 1) * BPC, :]).reshape(
            NT, TOK, H
        )
        packf = base_packf.copy()
        b_of_p = c * BPC + (p % BPC)
        s_full = SS * t_idx[None, :] + s_of_p[:, None]  # [128, NT]
        valid = s_full < lens[b_of_p][:, None]
        packf[:, OFF_MASK : OFF_MASK + NT] = np.where(valid, 0.0, MASK_NEG)
        in_maps.append({"x": xc, "packf": packf, "packb": packb})
    return in_maps


def run(hidden_states, Wq, bq, Wk, bk, lengths, trace=False):
    """Run on 8 cores; returns (output [B, H] fp32, BassKernelResults)."""
    nc, _ = _get_program()
    in_maps = _make_in_maps(hidden_states, Wq, bq, Wk, bk, lengths)
    res = run_bass_kernel_spmd(
        nc, in_maps, core_ids=list(range(NCORES)), trace=trace
    )
    out = np.concatenate([np.asarray(r["y"]) for r in res.results], axis=0)
    return out.astype(np.float32), res


def kernel(hidden_states, Wq, bq, Wk, bk, lengths):
    out, _ = run(hidden_states, Wq, bq, Wk, bk, lengths)
    return out


# ---------------------------------------------------------------------------
# Benchmarking helpers (not used by the grader's kernel() path)
# ---------------------------------------------------------------------------


def _make_sharded_callable(nc, in_maps):
    """Replicate run_bass_via_pjrt's multi-core path, but return a reusable
    jitted callable + device-resident inputs so repeat timing excludes
    host->device transfer of the big operands."""
    import jax
    import concourse.mybir as mybir_
    from jax.experimental.shard_map import shard_map
    from jax.sharding import Mesh, NamedSharding, PartitionSpec

    from concourse import bass2jax

    bass2jax.install_neuronx_cc_hook()
    n_cores = len(in_maps)
    partition_name = (
        nc.partition_id_tensor.name if nc.partition_id_tensor else None
    )
    in_names, out_names, out_avals, zero_outs = [], [], [], []
    for alloc in nc.m.functions[0].allocations:
        if not isinstance(mybir_.MemoryLocationSet, type) or not isinstance(
            alloc, mybir_.MemoryLocationSet
        ):
            continue
        if not alloc.memorylocations:
            continue
        name = alloc.memorylocations[0].name
        if alloc.kind == "ExternalInput":
            if name != partition_name:
                in_names.append(name)
        elif alloc.kind == "ExternalOutput":
            shape = tuple(alloc.tensor_shape)
            dtype = mybir_.dt.np(alloc.dtype)
            out_names.append(name)
            out_avals.append(jax.core.ShapedArray(shape, dtype))
            zero_outs.append(np.zeros(shape, dtype))
    n_params = len(in_names)
    all_names = in_names + out_names
    if partition_name is not None:
        all_names = all_names + [partition_name]

    def _body(*args):
        operands = list(args)
        if partition_name is not None:
            operands.append(bass2jax.partition_id_tensor())
        outs = bass2jax._bass_exec_p.bind(
            *operands,
            out_avals=tuple(out_avals),
            in_names=tuple(all_names),
            out_names=tuple(out_names),
            lowering_input_output_aliases=(),
            sim_require_finite=True,
            sim_require_nnan=True,
            nc=nc,
        )
        return tuple(outs)

    devices = jax.devices()[:n_cores]
    mesh = Mesh(np.asarray(devices), ("core",))
    nout = len(out_names)
    donate = tuple(range(n_params, n_params + nout))
    sharded = jax.jit(
        shard_map(
            _body,
            mesh=mesh,
            in_specs=(PartitionSpec("core"),) * (n_params + nout),
            out_specs=(PartitionSpec("core"),) * nout,
            check_rep=False,
        ),
        donate_argnums=donate,
        keep_unused=True,
    )
    sh = NamedSharding(mesh, PartitionSpec("core"))
    dev_in = [
        jax.device_put(
            np.concatenate([np.asarray(m[name]) for m in in_maps], axis=0), sh
        )
        for name in in_names
    ]
    concat_zero_shapes = [
        ((n_cores * z.shape[0], *z.shape[1:]), z.dtype) for z in zero_outs
    ]

    def call():
        zs = [np.zeros(s, d) for s, d in concat_zero_shapes]
        outs = sharded(*dev_in, *zs)
        for o in outs:
            o.block_until_ready()
        return outs

    return call


def bench_loop(hidden_states, Wq, bq, Wk, bk, lengths, reps=(1, 11, 51), iters=6):
    """Estimate device exec time by running the NEFF `n` times inside one
    dispatch for several n and fitting the slope (ns per execution)."""
    import time

    import jax
    from jax.experimental.shard_map import shard_map
    from jax.sharding import Mesh, NamedSharding, PartitionSpec

    import concourse.mybir as mybir_
    from concourse import bass2jax

    nc, _ = _get_program()
    in_maps = _make_in_maps(hidden_states, Wq, bq, Wk, bk, lengths)
    bass2jax.install_neuronx_cc_hook()
    n_cores = len(in_maps)
    partition_name = nc.partition_id_tensor.name if nc.partition_id_tensor else None
    in_names, out_names, out_avals = [], [], []
    for alloc in nc.m.functions[0].allocations:
        if not isinstance(alloc, mybir_.MemoryLocationSet) or not alloc.memorylocations:
            continue
        name = alloc.memorylocations[0].name
        if alloc.kind == "ExternalInput":
            if name != partition_name:
                in_names.append(name)
        elif alloc.kind == "ExternalOutput":
            out_names.append(name)
            out_avals.append(
                jax.core.ShapedArray(tuple(alloc.tensor_shape), mybir_.dt.np(alloc.dtype))
            )
    all_names = in_names + out_names
    if partition_name is not None:
        all_names = all_names + [partition_name]

    devices = jax.devices()[:n_cores]
    mesh = Mesh(np.asarray(devices), ("core",))
    sh = NamedSharding(mesh, PartitionSpec("core"))
    dev_in = [
        jax.device_put(
            np.concatenate([np.asarray(m[name]) for m in in_maps], axis=0), sh
        )
        for name in in_names
    ]
    dev_in += [
        jax.device_put(
            np.zeros((n_cores * a.shape[0], *a.shape[1:]), a.dtype), sh
        )
        for a in out_avals
    ]

    nin = len(in_names)
    nout = len(out_names)

    def make_fn(n):
        def body_n(*args):
            ins, zs = args[:nin], args[nin:]
            outs = None
            for _ in range(n):
                operands = list(ins) + list(zs)
                if partition_name is not None:
                    operands.append(bass2jax.partition_id_tensor())
                outs = bass2jax._bass_exec_p.bind(
                    *operands,
                    out_avals=tuple(out_avals),
                    in_names=tuple(all_names),
                    out_names=tuple(out_names),
                    lowering_input_output_aliases=(),
                    sim_require_finite=True,
                    sim_require_nnan=True,
                    nc=nc,
                )
            return tuple(outs)

        return jax.jit(
            shard_map(
                body_n,
                mesh=mesh,
                in_specs=(PartitionSpec("core"),) * (nin + nout),
                out_specs=(PartitionSpec("core"),) * nout,
                check_rep=False,
            )
        )

    results = {}
    for n in reps:
        fn = make_fn(n)
        outs = fn(*dev_in)
        for o in outs:
            o.block_until_ready()
        ts = []
        for _ in range(iters):
            t0 = time.perf_counter()
            outs = fn(*dev_in)
            for o in outs:
                o.block_until_ready()
            ts.append(time.perf_counter() - t0)
        results[n] = min(ts)
    ns = sorted(results)
    slope = (results[ns[-1]] - results[ns[0]]) / (ns[-1] - ns[0])
    return results, slope


def bench(hidden_states, Wq, bq, Wk, bk, lengths, iters=20):
    """Returns (list of per-iter wall seconds, overhead estimate seconds)."""
    import time

    nc, _ = _get_program()
    in_maps = _make_in_maps(hidden_states, Wq, bq, Wk, bk, lengths)
    call = _make_sharded_callable(nc, in_maps)
    call()  # warm/compile
    times = []
    for _ in range(iters):
        t0 = time.perf_counter()
        call()
        times.append(time.perf_counter() - t0)

    # dispatch-overhead floor: trivial kernel doing one small DMA
    if "nc_trivial" not in _CACHE:
        ncT = bacc.Bacc(None, target_bir_lowering=False, debug=False)
        a = ncT.dram_tensor("a", [BPC, H], F32, kind="ExternalInput").ap()
        yT = ncT.dram_tensor("y", [BPC, H], F32, kind="ExternalOutput").ap()
        with tile.TileContext(ncT) as tcT:
            with tcT.tile_pool(name="p", bufs=1) as pool:
                tt = pool.tile([BPC, H], F32)
                ncT.sync.dma_start(tt, a)
                ncT.sync.dma_start(yT, tt)
        ncT.finalize()
        _CACHE["nc_trivial"] = ncT
    ncT = _CACHE["nc_trivial"]
    triv_maps = [{"a": np.zeros((BPC, H), np.float32)} for _ in range(NCORES)]
    tcall = _make_sharded_callable(ncT, triv_maps)
    tcall()
    otimes = []
    for _ in range(iters):
        t0 = time.perf_counter()
        tcall()
        otimes.append(time.perf_counter() - t0)
    return times, min(otimes)



# revision 23
# speedup vs baseline: 1.4267x; 1.4267x over previous
"""Trainium2 Bass kernel for nn_Attention_82660940579436.

Computation (see reference):
    q     = mean_s(hidden @ Wq.T + bq)            [B, H]
    key   = tanh(hidden @ Wk.T + bk)              [S, B, H]
    score = einsum('bsh,bh->bs', key, q) + mask   [B, S]
    out   = softmax(score) @ key                  [B, H]

Sharding: data-parallel over batch. B=32 over 8 cores -> 4 batches/core.

Fused streaming design (single pass, software-pipelined emission):
  - h tiles [128 tok=(s,g), 512 j] stream in via SWDGE (fp32->bf16 cast),
    8 tiles per DMA, 12-slab ring.
  - q-sum runs as tiny rank-4 PE matmuls on the UNtransposed h tiles
    (out [128 j, (c,g)] accumulated in PSUM over all tiles) -- q is ready
    right after the last load (~48us), not after the last key matmul.
  - hT: first NX tiles via PE transpose + DVE copy; the rest via the DMA
    XBAR transpose (SBUF->SBUF, 2 tiles per instruction). The xbar emits
    a fixed token permutation (position m holds token 2*(m%64)+m//64 of
    its tile); the permutation is absorbed into the constant packs (mask
    columns, ind4, qrep) -- every downstream op is token-parallel or a
    permutation-invariant contraction over tokens.
  - key matmul: bf16, moving operand = Wk chunks (4 x 512 cols); bias via
    a single fp8e4 DoubleRow matmul (K=1, 0.5 cycles/col, exact for these
    operands -- verified on HW).
  - tanh on ACT over 2-tile PSUM batches -> keys ring.
  - score pipeline trails by LAG tiles (the q barrier): prod on DVE/Pool,
    row-reduce on DVE (2-tile) / ACT (copy+accum), exp with the mask as a
    per-partition bias, ei = ind4*e (DVE TSP), numer/den as rank-4/1 PE
    matmuls accumulated in PSUM ([128 i, (c,g)] resp. [1, 4]).
  - epilogue: rcp(den), replicate via tiny matmul, scale, DMA out
    [128, (c,g)]; host reorders to [B, H].

fp8 for the big matmul was measured (numpy) at rel_err 3.9e-2 -- over the
2e-2 gate -- so the key matmul stays bf16.
"""

import sys
from contextlib import ExitStack

import numpy as np

if "/opt/trn_rl_repo" not in sys.path:
    sys.path.insert(0, "/opt/trn_rl_repo")

import ml_dtypes  # noqa: E402

import concourse.bacc as bacc  # noqa: E402
import concourse.mybir as mybir  # noqa: E402
import concourse.tile as tile  # noqa: E402
from concourse.bass_utils import run_bass_kernel_spmd  # noqa: E402

S, B, H = 4096, 32, 512
NCORES = 8
BPC = B // NCORES  # 4 batches per core
NT = 128  # tiles per core
SS = S // NT  # 32 s-positions per tile
TOK = SS * BPC  # 128 tokens per tile
HC = H // 128  # 4 chunks of the H (j / i) dims
MASK_NEG = -60.0
F32 = mybir.dt.float32
BF16 = mybir.dt.bfloat16
FP8 = mybir.dt.float8e4
AF = mybir.ActivationFunctionType
ALU = mybir.AluOpType
PM = mybir.MatmulPerfMode
BF16NP = ml_dtypes.bfloat16
FP8NP = ml_dtypes.float8_e4m3

# ---- tuning knobs ----
KNOBS = {
    "NX": 40,  # tiles using PE transpose; rest use xbar DMA transpose
    "LAG": 44,  # score pipeline trails key pipeline by this many tiles
    "KQ": 45,  # emit q computation after this key tile
    "H_BUFS": 24,  # h ring slabs (LOADS_PER_DMA tiles each)
    "K_BUFS": 28,  # keys ring (2 tiles each)
    "HT_BUFS": 1,  # hT sbuf bufs (PE-transposed tiles)
    "HTX_BUFS": 4,  # hT sbuf bufs (xbar pairs)
    "PROD_DVE": (11, 20),  # prod on DVE for t%20 < 11, else gpsimd
    "RED_DVE": (7, 10),  # reduce on DVE for pair%10 < 7, else ACT
    "BIAS_FP8": False,  # DR matmuls corrupt interleaved PSUM accumulation on HW
    "LOADS_PER_DMA": 4,
    "DEBUG": False,
}

# fp32 const pack layout (offsets in fp32 elements, [128, PACKF] tensor)
OFF_MASK = 0  # [128, NT] mask bias (0 / MASK_NEG), col=tile (perm-aware)
OFF_BQ = NT  # [4, 512] bq rows
OFF_ONEROW_F = NT + 512  # [1, 128] ones (fp32) for rcp_rep matmul
OFF_ZERO = NT + 640  # [128, 1] zeros (tanh bias)
PACKF = NT + 641

# bf16 const pack layout ([128, PACKB])
OFFB_WK = 0  # [128, 2048] WkT chunks
OFFB_WQ = 2048  # [128, 2048] (WqT/S) chunks
OFFB_ID = 4096  # [128, 128] identity
OFFB_I4_NAT = 4224  # [128, 4] indicator
OFFB_I4_PERM = 4228  # [128, 4] indicator (xbar permuted)
OFFB_ONES1 = 4232  # [128, 1] ones
OFFB_ZROW = 4233  # [1, 23] zeros (zero-init matmuls)
OFFB_BKROW = 4256  # [1, 512] bk (bf16 bias fallback)
OFFB_ONEROW = 4768  # [1, 128] ones row
OFFB_I4T_NAT = 4896  # [4, 128] indicator transposed (bf16)
OFFB_I4T_PERM = 5024  # [4, 128] indicator transposed, permuted (bf16)
PACKB = 5152

# fp8 pack ([1, PACK8]): DoubleRow bias operands
OFF8_L = 0  # [1, 256] lhsT pairs: slot0 = ones(128), slot1 = zeros
OFF8_R = 256  # [1, 1024] rhs pairs: slot0 = bk, slot1 = zeros
PACK8 = 1280


def _xbar_perm():
    """Token permutation of the xbar output: position m holds token 2*(m%64)+m//64."""
    m = np.arange(128)
    return 2 * (m % 64) + m // 64


def _build_kernel_body(tc, aps):
    nc = tc.nc
    x, packf, packb, pack8 = aps["x"], aps["packf"], aps["packb"], aps["pack8"]
    y_num = aps["y_num"]
    dbg = KNOBS["DEBUG"]

    NX = KNOBS["NX"]
    LAG = KNOBS["LAG"]
    KQ = KNOBS["KQ"]
    LPD = KNOBS["LOADS_PER_DMA"]
    NB = NT // LPD  # load batches
    HB = KNOBS["H_BUFS"]

    with ExitStack() as ctx:
        consts = ctx.enter_context(tc.tile_pool(name="consts", bufs=1))
        ph = ctx.enter_context(tc.tile_pool(name="h", bufs=HB))
        phT = ctx.enter_context(tc.tile_pool(name="hT", bufs=KNOBS["HT_BUFS"]))
        phTx = ctx.enter_context(tc.tile_pool(name="hTx", bufs=KNOBS["HTX_BUFS"]))
        pkeys = ctx.enter_context(tc.tile_pool(name="keys", bufs=KNOBS["K_BUFS"]))
        pprod = ctx.enter_context(tc.tile_pool(name="prod", bufs=4))
        psmall = ctx.enter_context(tc.tile_pool(name="small", bufs=6))
        pout = ctx.enter_context(tc.tile_pool(name="out", bufs=1))
        pps_q = ctx.enter_context(tc.tile_pool(name="ps_q", bufs=1, space="PSUM"))
        pps_key = ctx.enter_context(tc.tile_pool(name="ps_key", bufs=2, space="PSUM"))
        pps_hT = ctx.enter_context(tc.tile_pool(name="ps_hT", bufs=2, space="PSUM"))
        pps_d = ctx.enter_context(tc.tile_pool(name="ps_d", bufs=1, space="PSUM"))

        # ---- constants: one DMA per pack ----
        cf = consts.tile([128, PACKF], F32)
        nc.sync.dma_start(cf, packf)
        cb = consts.tile([128, PACKB], BF16)
        nc.sync.dma_start(cb, packb)
        c8 = consts.tile([1, PACK8], FP8)
        nc.sync.dma_start(c8, pack8)

        def wk_sb(c):
            return cb[:, OFFB_WK + c * 512 : OFFB_WK + (c + 1) * 512]

        def wq_sb(c):
            return cb[:, OFFB_WQ + c * 512 : OFFB_WQ + (c + 1) * 512]

        id_sb = cb[:, OFFB_ID : OFFB_ID + 128]
        maskb_sb = cf[:, OFF_MASK : OFF_MASK + NT]
        bq_sb = cf[0:BPC, OFF_BQ : OFF_BQ + H]
        onerow_f = cf[0:1, OFF_ONEROW_F : OFF_ONEROW_F + 128]
        zero_sb = cf[:, OFF_ZERO : OFF_ZERO + 1]
        i4_nat = cb[:, OFFB_I4_NAT : OFFB_I4_NAT + BPC]
        i4_perm = cb[:, OFFB_I4_PERM : OFFB_I4_PERM + BPC]
        ones1 = cb[:, OFFB_ONES1 : OFFB_ONES1 + 1]
        zrow = cb[0:1, OFFB_ZROW : OFFB_ZROW + 23]
        bkrow = cb[0:1, OFFB_BKROW : OFFB_BKROW + H]
        onerow_b = cb[0:1, OFFB_ONEROW : OFFB_ONEROW + 128]
        i4t_nat = cb[0:BPC, OFFB_I4T_NAT : OFFB_I4T_NAT + 128]
        i4t_perm = cb[0:BPC, OFFB_I4T_PERM : OFFB_I4T_PERM + 128]
        bias8_l = c8[:, OFF8_L : OFF8_L + 256].rearrange(
            "p (two m) -> p two m", two=2
        )
        bias8_r = c8[:, OFF8_R : OFF8_R + 1024].rearrange(
            "p (two n) -> p two n", two=2
        )

        # Shared PSUM banks: Tq (dummies then qacc), Td (q/qrep/den/rr chain).
        Tq = pps_q.tile([128, 512], F32, tag="qa")
        Td = pps_d.tile([128, 512], F32, tag="d")

        # Dummy PE ops observing each const-pack DMA lane once (walrus allows
        # only ONE sync-wait per Matmult).
        nc.tensor.matmul(
            Tq, bias8_l, bias8_r, start=True, stop=True, perf_mode=PM.DoubleRow
        )
        nc.tensor.matmul(
            Tq[:, 0:128], onerow_b, onerow_b, start=True, stop=True
        )
        nc.tensor.matmul(
            Tq[:, 128:256], onerow_f, cf[0:1, 0:128], start=True, stop=True
        )

        # qacc: [128 j_local, (c, g)] accumulated over all tiles; zero-init
        # matmul so per-tile qsum matmuls never need start=True.
        qacc_ps = Tq[:, 0 : HC * BPC]
        nc.tensor.matmul(
            qacc_ps, onerow_b, zrow[:, 0:16], start=True, stop=False,
            skip_group_check=True,
        )

        h_slabs = [None] * HB
        hT_nat = [None] * NT  # per-tile [128, 512] (PE transpose path)
        hTx_pairs = [None] * (NT // 2)  # per-pair [128, 1024] (xbar path)
        key_pairs = [None] * (NT // 2)
        pc_tile = pout.tile([TOK, H], BF16, tag="pc")  # ACT-reduce dump
        pair_bufs = {}
        if dbg:
            e_all = pout.tile([TOK, NT], F32, tag="e_all")
        else:
            e_all = None
        state = {
            "q_done": False,
            "qrep_nat": None,
            "qrep_perm": None,
            "numer_ps": None,
            "den_ps": None,
            "prod_pair": None,
            "sc_pair": None,
            "next_load": min(HB, NB),
            "s_prod": 0,
            "s_red": 0,
            "s_post": 0,
        }

        def h_tile(t):
            return h_slabs[(t // LPD) % HB][:, (t % LPD) * H : (t % LPD + 1) * H]

        def emit_load(b):
            slab = ph.tile([TOK, LPD * H], BF16, tag="h")
            h_slabs[b % HB] = slab
            nc.gpsimd.dma_start(
                slab, x[b * LPD : (b + 1) * LPD].rearrange("t p j -> p t j")
            )

        def emit_qsum(t):
            ht = h_tile(t)
            for c in range(HC):
                nc.tensor.matmul(
                    qacc_ps[:, c * BPC : (c + 1) * BPC],
                    ht[:, c * 128 : (c + 1) * 128],
                    i4_nat,
                    start=False,
                    stop=(t == NT - 1 and c == HC - 1),
                    skip_group_check=True,
                )

        def emit_transpose(t):
            hT_ps = pps_hT.tile([128, H], BF16, tag="hT")
            ht = h_tile(t)
            for c in range(HC):
                nc.tensor.transpose(
                    hT_ps[:, c * 128 : (c + 1) * 128],
                    ht[:, c * 128 : (c + 1) * 128],
                    id_sb,
                )
            hT_sb = phT.tile([128, H], BF16, tag="hT_sb")
            nc.vector.tensor_copy(hT_sb, hT_ps)
            hT_nat[t] = hT_sb

        def emit_xbar(t):
            # XBAR transpose of the pair (t, t+1). With a 3D out AP
            # [p, cb (stride 128), k (stride 1)] the xbar lands NATURALLY:
            # hTx[j, cb*128 + tok] = h[tok, cb*128 + j], cb = tp*4 + c.
            slab = h_slabs[(t // LPD) % HB]
            lo = (t % LPD) * H
            hTx = phTx.tile([128, 2 * H], BF16, tag="hTx")
            nc.sync.dma_start(
                hTx.rearrange("p (cb k) -> p cb k", k=128),
                slab[:, lo : lo + 2 * H],
                transpose=True,
            )
            hTx_pairs[t // 2] = hTx

        def lhsT_for(t, c):
            if t < NX:
                return hT_nat[t][:, c * 128 : (c + 1) * 128]
            hTx = hTx_pairs[t // 2]
            cb = (t % 2) * HC + c
            return hTx[:, cb * 128 : (cb + 1) * 128]

        def emit_keymm(t):
            tp = t % 2
            if tp == 0:
                kp = pps_key.tile([TOK, 2 * H], F32, tag="key")
                key_pairs[t // 2] = [kp, None]
            kp = key_pairs[t // 2][0]
            out = kp[:, tp * H : (tp + 1) * H]
            if KNOBS["BIAS_FP8"]:
                nc.tensor.matmul(
                    out, bias8_l, bias8_r, start=True, stop=False,
                    perf_mode=PM.DoubleRow, skip_group_check=True,
                )
            else:
                nc.tensor.matmul(
                    out, onerow_b, bkrow, start=True, stop=False,
                    skip_group_check=True,
                )
            for c in range(HC):
                nc.tensor.matmul(
                    out,
                    lhsT_for(t, c),
                    wk_sb(c),
                    start=False,
                    stop=(c == HC - 1),
                    skip_group_check=True,
                )

        def emit_tanh(t):
            kp = key_pairs[t // 2][0]
            keys = pkeys.tile([TOK, 2 * H], BF16, tag="keys")
            nc.scalar.activation(keys, kp, AF.Tanh, bias=zero_sb)
            key_pairs[t // 2][1] = keys

        def emit_q():
            qacc_sb = pout.tile([128, HC * BPC], BF16, tag="qacc_sb")
            nc.vector.tensor_copy(qacc_sb, qacc_ps)
            q_ps = Td[0:BPC, :]
            for c in range(HC):
                nc.tensor.matmul(
                    q_ps,
                    qacc_sb[:, c * BPC : (c + 1) * BPC],
                    wq_sb(c),
                    start=(c == 0),
                    stop=(c == HC - 1),
                )
            q_sb = pout.tile([BPC, H], BF16, tag="q_sb")
            nc.vector.tensor_add(q_sb, q_ps, bq_sb)
            for which, i4t in (("qrep_nat", i4t_nat), ("qrep_perm", i4t_perm)):
                qr_ps = Td
                nc.tensor.matmul(qr_ps, i4t, q_sb, start=True, stop=True)
                qr_sb = pout.tile([128, H], BF16, tag=which)
                nc.vector.tensor_copy(qr_sb, qr_ps)
                state[which] = qr_sb
                del qr_ps, qr_sb
            # reuse the qacc bank: its group stopped and it was copied out
            numer_ps = Tq[:, 0 : HC * BPC]
            nc.tensor.matmul(
                numer_ps, onerow_b, zrow[:, 0:16], start=True, stop=False,
                skip_group_check=True,
            )
            state["numer_ps"] = numer_ps
            state["q_done"] = True
            if dbg:
                nc.sync.dma_start(aps["d_qacc"], qacc_sb)
                nc.sync.dma_start(aps["d_q"], q_sb)
                nc.sync.dma_start(aps["d_qrep"], state["qrep_nat"])

        def stage_prod(s):
            qrep = state["qrep_nat"]
            keys = key_pairs[s // 2][1]
            tp = s % 2
            if tp == 0:
                prod_pair = pprod.tile([TOK, 2 * H], BF16, tag="prod")
                sc_pair = psmall.tile([TOK, 2], F32, tag="sc")
                pair_bufs[s // 2] = (prod_pair, sc_pair)
            prod = pair_bufs[s // 2][0]
            kslice = keys[:, tp * H : (tp + 1) * H]
            pslice = prod[:, tp * H : (tp + 1) * H]
            a, b_ = KNOBS["PROD_DVE"]
            if s % b_ < a:
                nc.vector.tensor_mul(pslice, kslice, qrep)
            else:
                nc.gpsimd.tensor_mul(pslice, kslice, qrep)

        def stage_reduce(s):
            if s % 2 == 0:
                return
            prod, sc = pair_bufs[s // 2]
            ra, rb = KNOBS["RED_DVE"]
            if (s // 2) % rb < ra:
                nc.vector.tensor_reduce(
                    sc,
                    prod.rearrange("p (two i) -> p two i", two=2),
                    axis=mybir.AxisListType.X,
                    op=ALU.add,
                )
            else:
                nc.scalar.activation(
                    pc_tile, prod[:, 0:H], AF.Copy, accum_out=sc[:, 0:1]
                )
                nc.scalar.activation(
                    pc_tile, prod[:, H : 2 * H], AF.Copy, accum_out=sc[:, 1:2]
                )

        def stage_post(s):
            if s % 2 == 0:
                return
            sc = pair_bufs[s // 2][1]
            for tt in (0, 1):
                si = s - 1 + tt
                e_t = psmall.tile([TOK, 1], F32, tag="e")
                nc.scalar.activation(
                    e_t, sc[:, tt : tt + 1], AF.Exp,
                    bias=maskb_sb[:, si : si + 1],
                )
                if dbg:
                    nc.vector.tensor_copy(e_all[:, si : si + 1], e_t)
                ei_t = psmall.tile([TOK, BPC], BF16, tag="ei")
                nc.vector.tensor_scalar_mul(ei_t, i4_nat, e_t)
                ks = key_pairs[si // 2][1][:, tt * H : (tt + 1) * H]
                for c in range(HC):
                    nc.tensor.matmul(
                        state["numer_ps"][:, c * BPC : (c + 1) * BPC],
                        ks[:, c * 128 : (c + 1) * 128],
                        ei_t,
                        start=False,
                        stop=(si == NT - 1 and c == HC - 1),
                        skip_group_check=True,
                    )
                den_ps = state["den_ps"]
                if den_ps is None:
                    den_ps = Td[0:1, 0:BPC]
                    state["den_ps"] = den_ps
                nc.tensor.matmul(
                    den_ps, ones1, ei_t,
                    start=(si == 0), stop=(si == NT - 1),
                    skip_group_check=True,
                )

        def emit_score_stages(k):
            if not state["q_done"]:
                return
            while state["s_prod"] < min(NT, k - LAG + 1):
                stage_prod(state["s_prod"])
                state["s_prod"] += 1
            while state["s_red"] < min(NT, state["s_prod"] - 2):
                stage_reduce(state["s_red"])
                state["s_red"] += 1
            while state["s_post"] < min(NT, state["s_red"] - 2):
                stage_post(state["s_post"])
                state["s_post"] += 1
            if k >= NT + LAG:  # flush
                while state["s_red"] < NT:
                    stage_reduce(state["s_red"])
                    state["s_red"] += 1
                while state["s_post"] < NT:
                    stage_post(state["s_post"])
                    state["s_post"] += 1

        # ---------- emission schedule ----------
        for b in range(min(HB, NB)):
            emit_load(b)

        qsum_done = 0
        for k in range(NT):
            while (
                state["next_load"] < NB
                and k >= (state["next_load"] - HB) * LPD + LPD
            ):
                emit_load(state["next_load"])
                state["next_load"] += 1
            target = min(NT, ((k + 1) * NT + KQ - 1) // KQ)
            while qsum_done < target:
                emit_qsum(qsum_done)
                qsum_done += 1
            emit_score_stages(k)
            if k < NX:
                emit_transpose(k)
            elif k % 2 == 0:
                emit_xbar(k)
            emit_keymm(k)
            if k % 2 == 1:
                emit_tanh(k)
            if k == KQ:
                while qsum_done < NT:
                    emit_qsum(qsum_done)
                    qsum_done += 1
                emit_q()
        for k in range(NT, NT + LAG + 5):
            emit_score_stages(k)

        # ---------- epilogue ----------
        if dbg:
            num_dbg = pout.tile([128, HC * BPC], F32, tag="num_dbg")
            nc.vector.tensor_copy(num_dbg, state["numer_ps"])
            nc.sync.dma_start(aps["d_num"], num_dbg)
            nc.sync.dma_start(aps["d_keys0"], key_pairs[0][1])
            nc.sync.dma_start(aps["d_keys60"], key_pairs[60][1])
            nc.sync.dma_start(aps["d_e"], e_all)
            nc.sync.dma_start(aps["d_htx"], hTx_pairs[30])
        rcp_sb = pout.tile([1, BPC], F32, tag="rcp")
        nc.vector.reciprocal(rcp_sb, state["den_ps"])
        rr_ps = Td[:, 0:BPC]
        nc.tensor.matmul(rr_ps, onerow_f, rcp_sb, start=True, stop=True)
        if dbg:
            nc.sync.dma_start(aps["d_rcp"], rcp_sb)
        rr_sb = pout.tile([128, BPC], F32, tag="rr_sb")
        nc.vector.tensor_copy(rr_sb, rr_ps)
        out_sb = pout.tile([128, HC * BPC], F32, tag="out_sb")
        for c in range(HC):
            nc.vector.tensor_mul(
                out_sb[:, c * BPC : (c + 1) * BPC],
                state["numer_ps"][:, c * BPC : (c + 1) * BPC],
                rr_sb,
            )
        nc.sync.dma_start(y_num, out_sb)


_CACHE = {}


def _fix_dma_waits(nc):
    """walrus's DMA_DIRECT2D lowering has ONE sync-wait slot. The SWDGE h
    loads sit on one queue (qPoolDynamic): descriptor generation is program-
    ordered and same-buffer writes cannot reorder, so the WAW (DMA-lane) wait
    is hardware-redundant. Drop it; keep WAR/engine waits. Then sanity-check
    remaining wait counts (DMACopy: 1, others: 2, Drain/EVSEM exempt)."""
    for b in nc.m.functions[0].blocks:
        for i in b.instructions:
            si = i.sync_info
            if si is None:
                continue
            waits = list(si.on_wait)
            if (
                type(i).__name__ == "InstDMACopy"
                and getattr(i, "queue", "") == "qPoolDynamic"
                and len(waits) >= 2
            ):
                lane = [w for w in waits if w.ant_name.startswith("DMASW")]
                eng = [w for w in waits if not w.ant_name.startswith("DMA")]
                if len(lane) >= 1 and len(lane) + len(eng) == len(waits):
                    out0 = i.outs[0]
                    name = getattr(getattr(out0, "bass_ap", None), "tensor", None)
                    name = getattr(name, "name", "")
                    if name.startswith(("h", "slab")):
                        si.on_wait = eng
                        waits = eng
            if type(i).__name__ in ("InstDrain", "InstEventSemaphore"):
                continue
            limit = 1 if type(i).__name__ == "InstDMACopy" else 2
            if len(waits) > limit:
                raise RuntimeError(
                    f"{i.name} {type(i).__name__} has {len(waits)} waits "
                    f"(> {limit}): {[(w.ant_name, w.wait_value) for w in waits]}"
                )


def _get_program():
    if "nc" in _CACHE:
        return _CACHE["nc"], _CACHE["aps"]
    nc = bacc.Bacc(None, target_bir_lowering=False, debug=False)
    aps = {
        "x": nc.dram_tensor("x", [NT, TOK, H], F32, kind="ExternalInput").ap(),
        "packf": nc.dram_tensor("packf", [128, PACKF], F32, kind="ExternalInput").ap(),
        "packb": nc.dram_tensor("packb", [128, PACKB], BF16, kind="ExternalInput").ap(),
        "pack8": nc.dram_tensor("pack8", [1, PACK8], FP8, kind="ExternalInput").ap(),
        "y_num": nc.dram_tensor(
            "y_num", [128, HC * BPC], F32, kind="ExternalOutput"
        ).ap(),
    }
    if KNOBS["DEBUG"]:
        aps["d_qacc"] = nc.dram_tensor("d_qacc", [128, 16], BF16, kind="ExternalOutput").ap()
        aps["d_q"] = nc.dram_tensor("d_q", [BPC, H], BF16, kind="ExternalOutput").ap()
        aps["d_qrep"] = nc.dram_tensor("d_qrep", [128, H], BF16, kind="ExternalOutput").ap()
        aps["d_num"] = nc.dram_tensor("d_num", [128, 16], F32, kind="ExternalOutput").ap()
        aps["d_keys0"] = nc.dram_tensor("d_keys0", [128, 1024], BF16, kind="ExternalOutput").ap()
        aps["d_keys60"] = nc.dram_tensor("d_keys60", [128, 1024], BF16, kind="ExternalOutput").ap()
        aps["d_rcp"] = nc.dram_tensor("d_rcp", [1, BPC], F32, kind="ExternalOutput").ap()
        aps["d_e"] = nc.dram_tensor("d_e", [TOK, NT], F32, kind="ExternalOutput").ap()
        aps["d_htx"] = nc.dram_tensor("d_htx", [128, 2 * H], BF16, kind="ExternalOutput").ap()
    with tile.TileContext(nc) as tc:
        _build_kernel_body(tc, aps)
    nc.finalize()
    _fix_dma_waits(nc)
    _CACHE["nc"] = nc
    _CACHE["aps"] = aps
    return nc, aps


def _make_in_maps(hidden_states, Wq, bq, Wk, bk, lengths):
    hidden = np.asarray(hidden_states, dtype=np.float32)
    Wq = np.asarray(Wq, dtype=np.float32)
    Wk = np.asarray(Wk, dtype=np.float32)
    bqv = np.asarray(bq, dtype=np.float32)
    bkv = np.asarray(bk, dtype=np.float32)
    lens = np.asarray(lengths).astype(np.int64)

    NX = KNOBS["NX"]
    p = np.arange(128)
    perm = _xbar_perm()

    packb = np.zeros((128, PACKB), dtype=BF16NP)
    packb[:, OFFB_WK : OFFB_WK + 2048] = (
        np.ascontiguousarray(Wk.T)
        .reshape(HC, 128, H)
        .transpose(1, 0, 2)
        .reshape(128, 2048)
        .astype(BF16NP)
    )
    packb[:, OFFB_WQ : OFFB_WQ + 2048] = (
        (np.ascontiguousarray(Wq.T) / S)
        .reshape(HC, 128, H)
        .transpose(1, 0, 2)
        .reshape(128, 2048)
        .astype(BF16NP)
    )
    packb[:, OFFB_ID : OFFB_ID + 128] = np.eye(128, dtype=BF16NP)
    packb[:, OFFB_I4_NAT : OFFB_I4_NAT + BPC] = (
        p[:, None] % BPC == np.arange(BPC)[None, :]
    ).astype(BF16NP)
    packb[:, OFFB_I4_PERM : OFFB_I4_PERM + BPC] = (
        perm[:, None] % BPC == np.arange(BPC)[None, :]
    ).astype(BF16NP)
    packb[:, OFFB_ONES1] = BF16NP(1.0)
    packb[0, OFFB_BKROW : OFFB_BKROW + H] = bkv.astype(BF16NP)
    packb[0, OFFB_ONEROW : OFFB_ONEROW + 128] = BF16NP(1.0)
    packb[0:BPC, OFFB_I4T_NAT : OFFB_I4T_NAT + 128] = (
        p[None, :] % BPC == np.arange(BPC)[:, None]
    ).astype(BF16NP)
    packb[0:BPC, OFFB_I4T_PERM : OFFB_I4T_PERM + 128] = (
        perm[None, :] % BPC == np.arange(BPC)[:, None]
    ).astype(BF16NP)

    pack8 = np.zeros((1, PACK8), dtype=FP8NP)
    pack8[0, OFF8_L : OFF8_L + 128] = FP8NP(1.0)
    pack8[0, OFF8_R : OFF8_R + H] = bkv.astype(FP8NP)

    base_packf = np.zeros((128, PACKF), dtype=np.float32)
    base_packf[0:BPC, OFF_BQ : OFF_BQ + H] = bqv[None, :]
    base_packf[0, OFF_ONEROW_F : OFF_ONEROW_F + 128] = 1.0

    in_maps = []
    t_idx = np.arange(NT)
    for core in range(NCORES):
        xc = np.ascontiguousarray(
            hidden[:, core * BPC : (core + 1) * BPC, :]
        ).reshape(NT, TOK, H)
        packf = base_packf.copy()
        tok_of_p = np.broadcast_to(p[:, None], (128, NT))
        b_of_p = core * BPC + tok_of_p % BPC
        s_full = SS * t_idx[None, :] + tok_of_p // BPC
        valid = s_full < lens[b_of_p]
        packf[:, OFF_MASK : OFF_MASK + NT] = np.where(valid, 0.0, MASK_NEG)
        in_maps.append({"x": xc, "packf": packf, "packb": packb, "pack8": pack8})
    return in_maps


def run(hidden_states, Wq, bq, Wk, bk, lengths, trace=False):
    """Run on 8 cores; returns (output [B, H] fp32, BassKernelResults)."""
    nc, _ = _get_program()
    in_maps = _make_in_maps(hidden_states, Wq, bq, Wk, bk, lengths)
    res = run_bass_kernel_spmd(
        nc, in_maps, core_ids=list(range(NCORES)), trace=trace
    )
    outs = []
    for r in res.results:
        ynum = np.asarray(r["y_num"])  # [128 i_local, (c, g)], already / den
        o = ynum.reshape(128, HC, BPC).transpose(2, 1, 0).reshape(BPC, H)
        outs.append(o)
    out = np.concatenate(outs, axis=0)
    return out.astype(np.float32), res


def kernel(hidden_states, Wq, bq, Wk, bk, lengths):
    out, _ = run(hidden_states, Wq, bq, Wk, bk, lengths)
    return out
